# revision 1
# baseline (speedup 1.0000x reference)
"""v3: dma_gather (CounterMachine) replaces 1600 indirect DMAs with 100.

Same math as kernel.py (see its docstring). Differences:
- Per type-bucket COMPACT tables: each 51200-edge bucket touches ~32045
  distinct endpoints (< 2^15), so local indices fit int16 and the whole
  2048-edge macro gathers in ONE InstDMAGatherAnt per side (u / v).
- Table rows are 64 f32 = 256B (dma_gather constraint); the LayerNorm
  stats (sx+sx', sqx+sqx') are precomputed per edge on the host and
  preloaded once as a [128, NMACRO*2G] f32 constant tile.
"""

import os
import numpy as np

N, E = 50000, 800000
C, NT, ET, H, D = 128, 8, 16, 64, 4
TOTAL_IN = 2 * C + 2 * NT + ET  # 288
EPS = 1e-5

P = 128
G = 16
EDGES_PER_MACRO = P * G     # 2048
NCORES = 8
TYPES_PER_CORE = ET // NCORES   # 2
TMACRO = 25
NMACRO = TYPES_PER_CORE * TMACRO  # 50
E_TYPE_PAD = TMACRO * EDGES_PER_MACRO   # 51200
E_PAD = NMACRO * EDGES_PER_MACRO        # 102400
NGROUPS = NMACRO * G        # 800
CTAB = 32768                # compact table rows per bucket
AW = 65                     # a | ones

_CACHE = {}
LAST_RESULTS = None


def _build_program():
    import concourse.bacc as bacc
    import concourse.bass as bass
    import concourse.tile as tile
    import concourse.mybir as mybir
    from concourse.masks import make_identity

    f32 = mybir.dt.float32
    i16 = mybir.dt.int16
    Alu = mybir.AluOpType
    Act = mybir.ActivationFunctionType

    nc = bacc.Bacc("TRN2", target_bir_lowering=False, debug=False,
                   num_devices=NCORES, dynamic_dma_scratch_size=65536)

    uc = nc.dram_tensor("uc", [TYPES_PER_CORE * CTAB, 64], f32,
                        kind="ExternalInput").ap()
    vc = nc.dram_tensor("vc", [TYPES_PER_CORE * CTAB, 64], f32,
                        kind="ExternalInput").ap()
    ridx = nc.dram_tensor("ridx", [P, NMACRO * P], i16,
                          kind="ExternalInput").ap()
    cidx = nc.dram_tensor("cidx", [P, NMACRO * P], i16,
                          kind="ExternalInput").ap()
    s12 = nc.dram_tensor("s12", [P, NMACRO * 2 * G], f32,
                         kind="ExternalInput").ap()
    cetrow = nc.dram_tensor("cetrow", [P, TYPES_PER_CORE * 64], f32,
                            kind="ExternalInput").ap()
    b0row = nc.dram_tensor("b0row", [P, 64], f32, kind="ExternalInput").ap()
    w2a = nc.dram_tensor("w2a", [AW, 16], f32, kind="ExternalInput").ap()
    irow = nc.dram_tensor("irow", [P, 16], f32, kind="ExternalInput").ap()
    out_d = nc.dram_tensor("out", [NMACRO, P, G * 16], f32,
                           kind="ExternalOutput").ap()

    with tile.TileContext(nc) as tc:
        with (
            tc.tile_pool(name="const", bufs=1) as constp,
            tc.tile_pool(name="gmac", bufs=3) as gpool,
            tc.tile_pool(name="amac", bufs=2) as apool,
            tc.tile_pool(name="atr", bufs=4) as atp,
            tc.tile_pool(name="stats", bufs=2) as stp,
            tc.tile_pool(name="expt", bufs=2) as expp,
            tc.tile_pool(name="outt", bufs=2) as outp,
            tc.tile_pool(name="pstr", bufs=4, space="PSUM") as ps_t,
            tc.tile_pool(name="pso", bufs=2, space="PSUM") as ps_o,
        ):
            idx_r = constp.tile([P, NMACRO * P], i16)
            idx_c = constp.tile([P, NMACRO * P], i16)
            nc.sync.dma_start(idx_r[:], ridx)
            nc.sync.dma_start(idx_c[:], cidx)
            st12 = constp.tile([P, NMACRO * 2 * G], f32)
            nc.sync.dma_start(st12[:], s12)

            # ---- all edge scalars in ONE batched pass: rstd [P, NGROUPS] ----
            st12v = st12[:].rearrange("p (m two g) -> p m two g", two=2, g=G)
            S1a = st12v[:, :, 0, :]
            S2a = st12v[:, :, 1, :]
            m_a = constp.tile([P, NGROUPS], f32)
            q_a = constp.tile([P, NGROUPS], f32)
            m_a3 = m_a[:].rearrange("p (m g) -> p m g", g=G)
            q_a3 = q_a[:].rearrange("p (m g) -> p m g", g=G)
            nc.vector.tensor_scalar(m_a3, S1a, 1.0 / TOTAL_IN,
                                    3.0 / TOTAL_IN, Alu.mult, Alu.add)
            nc.vector.tensor_scalar(q_a3, S2a, 1.0 / TOTAL_IN,
                                    3.0 / TOTAL_IN + EPS, Alu.mult, Alu.add)
            nc.vector.tensor_tensor(m_a[:], m_a[:], m_a[:], Alu.mult)
            nc.vector.tensor_tensor(q_a[:], q_a[:], m_a[:], Alu.subtract)
            rstd_a = constp.tile([P, NGROUPS], f32)
            nc.scalar.sqrt(rstd_a[:], q_a[:])
            nc.vector.reciprocal(rstd_a[:], rstd_a[:])
            w2a_t = constp.tile([AW, 16], f32)
            nc.sync.dma_start(w2a_t[:], w2a)
            cet_t = constp.tile([P, TYPES_PER_CORE * 64], f32)
            nc.sync.dma_start(cet_t[:], cetrow)
            b0_t = constp.tile([P, 64], f32)
            nc.sync.dma_start(b0_t[:], b0row)
            irow_t = constp.tile([P, 16], f32)
            nc.sync.dma_start(irow_t[:], irow)
            id_t = constp.tile([P, P], f32)
            make_identity(nc, id_t[:])

            def mid_bc(ap2, n):
                (ps, pc), (fs, fc) = ap2.ap
                return bass.AP(ap2.tensor, ap2.offset,
                               [[ps, pc], [0, n], [fs, fc]])

            def bc(ap2, n):
                return bass.AP(ap2.tensor, ap2.offset,
                               list(ap2.ap) + [[0, n]])

            b0_bc3 = mid_bc(b0_t[:], G)
            irow_bc3 = mid_bc(irow_t[:], G)

            for m in range(NMACRO):
                tloc = m // TMACRO
                u_slice = uc[tloc * CTAB:(tloc + 1) * CTAB, :]
                v_slice = vc[tloc * CTAB:(tloc + 1) * CTAB, :]

                gu = gpool.tile([P, G * 64], f32, tag="gu")
                gv = gpool.tile([P, G * 64], f32, tag="gv")
                gu3 = gu[:].rearrange("p (g w) -> p g w", w=64)
                gv3 = gv[:].rearrange("p (g w) -> p g w", w=64)
                # chunk at 1024 idxs: stay within the SWDGE descriptor ring
                # and legal packet sizes (single_packet=False).
                CH = 1024
                for k0 in range(0, EDGES_PER_MACRO, CH):
                    g0 = k0 // P          # first group of this chunk
                    gn = CH // P          # groups per chunk
                    isl = slice(m * P + k0 // 16, m * P + (k0 + CH) // 16)
                    nc.gpsimd.dma_gather(
                        gu3[:, g0:g0 + gn, :], u_slice, idx_r[:, isl],
                        CH, CH, 64, single_packet=False)
                    nc.gpsimd.dma_gather(
                        gv3[:, g0:g0 + gn, :], v_slice, idx_c[:, isl],
                        CH, CH, 64, single_packet=False)
                nc.vector.tensor_tensor(gu[:], gu[:], gv[:], Alu.add)

                # ---- a = relu(rstd * g + b0)  (cet~ folded into tables) ----
                s_rstd = rstd_a[:, m * G:(m + 1) * G]
                a = apool.tile([P, G * AW], f32)
                a3 = a[:].rearrange("p (g w) -> p g w", w=AW)
                av = a3[:, :, 0:64]
                nc.vector.tensor_tensor(av, gu3, bc(s_rstd, 64), Alu.mult)
                nc.vector.tensor_tensor(av, av, b0_bc3, Alu.add)
                nc.vector.memset(a3[:, :, 64], 1.0)
                nc.scalar.activation(av, av, Act.Relu)

                # ---- per group: PE transpose, copy, W2 matmul ----
                ops = ps_o.tile([P, G * 16], f32)
                for gi in range(G):
                    at_ps = ps_t.tile([AW, P], f32)
                    nc.tensor.transpose(at_ps[:], a3[:, gi, :], id_t[:])
                    at_sb = atp.tile([AW, P], f32)
                    nc.scalar.copy(at_sb[:], at_ps[:])
                    nc.tensor.matmul(ops[:, gi * 16:(gi + 1) * 16],
                                     lhsT=at_sb[:], rhs=w2a_t[:],
                                     start=True, stop=True)

                # ---- batched softmax tail ----
                ex = expp.tile([P, G * 16], f32)
                nc.scalar.activation(ex[:], ops[:], Act.Exp)
                ex3 = ex[:].rearrange("p (r w) -> p r w", w=4)
                sums = stp.tile([P, 4 * G], f32)
                nc.vector.tensor_reduce(sums[:], ex3, mybir.AxisListType.X,
                                        Alu.add)
                rec = stp.tile([P, 4 * G], f32)
                nc.vector.reciprocal(rec[:], sums[:])
                ot = outp.tile([P, G * 16], f32)
                ot3 = ot[:].rearrange("p (r w) -> p r w", w=4)
                nc.vector.tensor_tensor(ot3, ex3, bc(rec[:], 4), Alu.mult)
                otg = ot[:].rearrange("p (g w) -> p g w", w=16)
                nc.vector.tensor_tensor(otg, irow_bc3, otg, Alu.subtract)
                nc.sync.dma_start(out_d[m], ot[:])

    nc.compile()
    return nc


def _prep_host(x, edge_index, edge_types, node_types, ln_w, ln_b, W1, b1, W2, b2):
    x = np.asarray(x, np.float32)
    ln_w = np.asarray(ln_w, np.float32)
    ln_b = np.asarray(ln_b, np.float32)
    W1 = np.asarray(W1, np.float32)
    b1 = np.asarray(b1, np.float32)
    W2 = np.asarray(W2, np.float32)
    b2 = np.asarray(b2, np.float32)

    W1p = ln_w[:, None] * W1
    s = W1p.sum(0)
    b0 = b1 + ln_b @ W1
    A = W1p[0:C]
    B = W1p[C:2 * C]
    C1 = W1p[2 * C:2 * C + NT]
    C2 = W1p[2 * C + NT:2 * C + 2 * NT]
    Cet = W1p[2 * C + 2 * NT:]
    cet_r = Cet - (3.0 / TOTAL_IN) * s[None, :]

    sx = x.sum(1)
    sqx = (x * x).sum(1)
    nt = np.asarray(node_types).astype(np.int64)
    mu_term = (sx / TOTAL_IN)[:, None] * s[None, :]
    u64 = (x @ A + C1[nt] - mu_term).astype(np.float32)
    v64 = (x @ B + C2[nt] - mu_term).astype(np.float32)

    w2a = np.concatenate([W2, b2[None, :]], 0).astype(np.float32)
    b0row = np.tile(b0[None, :].astype(np.float32), (P, 1))
    irow = np.tile(np.eye(D, dtype=np.float32).reshape(1, 16), (P, 1))

    row = np.asarray(edge_index[0]).astype(np.int64)
    col = np.asarray(edge_index[1]).astype(np.int64)
    et = np.asarray(edge_types).astype(np.int64)

    order = np.argsort(et, kind="stable")
    counts = np.bincount(et, minlength=ET)
    assert counts.max() <= E_TYPE_PAD, counts.max()
    starts = np.zeros(ET + 1, np.int64)
    np.cumsum(counts, out=starts[1:])

    def seq_to_gather_layout(vals, dtype):
        # edge slot (m, p, g) = seq m*2048 + p*16 + g -> list pos g*128+p
        # -> idx16[(pos%16 -> partition row), pos//16], replicated to 128.
        v = vals.reshape(NMACRO, P, G).transpose(0, 2, 1).reshape(NMACRO, 2048)
        # v[m, i] = list element i (i = g*128+p)
        pat = v.reshape(NMACRO, P, 16).transpose(0, 2, 1).reshape(NMACRO, 16, P)
        # pat[m, p16, s] = list[s*16+p16]
        full = np.tile(pat, (1, 8, 1))            # [NMACRO, 128, 128]
        return np.ascontiguousarray(
            full.transpose(1, 0, 2).reshape(P, NMACRO * P)).astype(dtype)

    def stats_layout(vals):
        v = vals.reshape(NMACRO, P, G).transpose(1, 0, 2).reshape(P, NGROUPS)
        return v  # [P, NMACRO*G], per-macro col block m*G..

    in_maps = []
    unscatter = []
    for c in range(NCORES):
        seq = np.zeros(E_PAD, np.int64)
        un = []
        ucs, vcs = [], []
        rloc = np.zeros(E_PAD, np.int64)
        cloc = np.zeros(E_PAD, np.int64)
        for k in range(TYPES_PER_CORE):
            t = c * TYPES_PER_CORE + k
            ids = order[starts[t]:starts[t + 1]]
            sl = slice(k * E_TYPE_PAD, k * E_TYPE_PAD + len(ids))
            seq[sl] = ids
            un.append((ids, k))
            bsl = slice(k * E_TYPE_PAD, (k + 1) * E_TYPE_PAD)
            br, bcol = row[seq[bsl]], col[seq[bsl]]
            uniq_r = np.unique(br)
            uniq_c = np.unique(bcol)
            assert len(uniq_r) <= CTAB and len(uniq_c) <= CTAB, (
                len(uniq_r), len(uniq_c))
            ut = np.zeros((CTAB, 64), np.float32)
            vt = np.zeros((CTAB, 64), np.float32)
            ut[:len(uniq_r)] = u64[uniq_r] + 0.5 * cet_r[t]
            vt[:len(uniq_c)] = v64[uniq_c] + 0.5 * cet_r[t]
            ucs.append(ut)
            vcs.append(vt)
            rloc[bsl] = np.searchsorted(uniq_r, br)
            cloc[bsl] = np.searchsorted(uniq_c, bcol)

        cetrow = np.tile(
            cet_r[c * TYPES_PER_CORE:(c + 1) * TYPES_PER_CORE].reshape(
                1, TYPES_PER_CORE * 64), (P, 1)).astype(np.float32)

        S1 = (sx[row[seq]] + sx[col[seq]]).astype(np.float32)
        S2 = (sqx[row[seq]] + sqx[col[seq]]).astype(np.float32)
        s1l = stats_layout(S1)
        s2l = stats_layout(S2)
        s12a = np.zeros((P, NMACRO * 2 * G), np.float32)
        for m in range(NMACRO):
            s12a[:, m * 2 * G:m * 2 * G + G] = s1l[:, m * G:(m + 1) * G]
            s12a[:, m * 2 * G + G:m * 2 * G + 2 * G] = s2l[:, m * G:(m + 1) * G]

        in_maps.append({
            "uc": np.concatenate(ucs, 0), "vc": np.concatenate(vcs, 0),
            "ridx": seq_to_gather_layout(rloc, np.int16),
            "cidx": seq_to_gather_layout(cloc, np.int16),
            "s12": s12a, "cetrow": cetrow, "b0row": b0row,
            "w2a": w2a, "irow": irow,
        })
        unscatter.append(un)
    return in_maps, unscatter


def kernel(**inputs) -> np.ndarray:
    global LAST_RESULTS
    from concourse.bass_utils import run_bass_kernel_spmd

    if "nc" not in _CACHE:
        _CACHE["nc"] = _build_program()
    nc = _CACHE["nc"]

    in_maps, unscatter = _prep_host(**{k: inputs[k] for k in
                                       ("x", "edge_index", "edge_types",
                                        "node_types", "ln_w", "ln_b", "W1",
                                        "b1", "W2", "b2")})

    res = run_bass_kernel_spmd(nc, in_maps, core_ids=list(range(NCORES)))
    LAST_RESULTS = res

    full = np.empty((E, 16), np.float32)
    for c in range(NCORES):
        rows = res.results[c]["out"].reshape(E_PAD, 16)
        for ids, k in unscatter[c]:
            full[ids] = rows[k * E_TYPE_PAD:k * E_TYPE_PAD + len(ids)]
    return full.reshape(E, D, D)



# revision 12
# speedup vs baseline: 22.3193x; 22.3193x over previous
"""v6: wire-optimized edge-parallel sheaf learner.

End-to-end wall time is dominated by host<->device transfer over the
axon tunnel (~50 MB/s), not on-device compute. v3 shipped ~302 MB per
call; v6 ships ~18 MB in + 13 MB out:

- Per-node tables (u64|sx|sqx bf16) ship PACKED [rows, 66] and SHARDED
  1/8 per core; each core expands its shard to 128-wide rows (the
  256B-aligned row pitch dma_gather requires) via an SBUF bounce, then
  an on-device AllGather over NeuronLink assembles the global tables.
- dma_gather indices are int16-limited (<32768), so the 51200-row table
  is split in two 25600-row chunks with zero-filled dummy rows; each
  edge gathers from both chunks (out-of-chunk index -> zero row) and
  the two results are summed. No masks needed.
- Only ONE uint16 global row-id per edge side ships (as [16, X], the
  native wrapped layout); both per-chunk int16 index tiles are derived
  on-device in f32 (exact for ints < 2^24) and replicated to 128
  partitions with SBUF DMAs.
- The softmax ships as uint8 att*255 (f32->int cast is round-to-nearest
  -even, verified on HW); the host reconstructs I - att. Quantization
  error ~0.2% of output scale vs the 2e-2 gate.
- The jitted shard_map executable is cached across calls; donated
  output buffers are zero-filled ON DEVICE; big inputs are device_put
  asynchronously as soon as host prep produces them; d2h is fetched
  per-shard and overlapped with the host-side unscatter.
"""

import numpy as np

N, E = 50000, 800000
C, NT, ET, H, D = 128, 8, 16, 64, 4
TOTAL_IN = 2 * C + 2 * NT + ET  # 288
EPS = 1e-5

P = 128
G = 16
EDGES_PER_MACRO = P * G     # 2048
NCORES = 8
TYPES_PER_CORE = ET // NCORES   # 2
TMACRO = 25
NMACRO = TYPES_PER_CORE * TMACRO  # 50
E_TYPE_PAD = TMACRO * EDGES_PER_MACRO   # 51200
E_PAD = NMACRO * EDGES_PER_MACRO        # 102400
AW = 65                     # a | ones

W = 128                     # table row width in HBM (bf16) = 256B
PK = 66                     # packed row width on the wire: u64|sx|sqx
SPLIT = 25000               # nodes per chunk
CHUNK = 25600               # rows per chunk (600 zero rows at the end)
DUMMY = 25000               # local zero-row index in each chunk
ROWS = 2 * CHUNK            # 51200 = 8 * 6400
SHARD_ROWS = ROWS // NCORES  # 6400
SB = SHARD_ROWS // P        # 50 row-blocks per shard

OSCALE = 255.0

_CACHE = {}
LAST_RESULTS = None


def _build_program():
    import concourse.bacc as bacc
    import concourse.bass as bass
    import concourse.tile as tile
    import concourse.mybir as mybir
    from concourse.masks import make_identity

    f32 = mybir.dt.float32
    bf16 = mybir.dt.bfloat16
    i16 = mybir.dt.int16
    u8 = mybir.dt.uint8
    Alu = mybir.AluOpType
    Act = mybir.ActivationFunctionType

    nc = bacc.Bacc("TRN2", target_bir_lowering=False, debug=False,
                   num_devices=NCORES, dynamic_dma_scratch_size=65536)

    tabu = nc.dram_tensor("tabu", [SHARD_ROWS, PK], bf16,
                          kind="ExternalInput").ap()
    tabv = nc.dram_tensor("tabv", [SHARD_ROWS, PK], bf16,
                          kind="ExternalInput").ap()
    ju = nc.dram_tensor("ju", [16, NMACRO * P], i16,
                        kind="ExternalInput").ap()
    jv = nc.dram_tensor("jv", [16, NMACRO * P], i16,
                        kind="ExternalInput").ap()
    cetrow = nc.dram_tensor("cetrow", [P, TYPES_PER_CORE * 64], f32,
                            kind="ExternalInput").ap()
    b0row = nc.dram_tensor("b0row", [P, 64], f32, kind="ExternalInput").ap()
    w2a = nc.dram_tensor("w2a", [AW, 16], f32, kind="ExternalInput").ap()
    out_d = nc.dram_tensor("out", [NMACRO, P, G * 16], u8,
                           kind="ExternalOutput").ap()

    with tile.TileContext(nc) as tc:
        with (
            tc.tile_pool(name="dram", bufs=1, space="DRAM") as dramp,
            tc.tile_pool(name="const", bufs=1) as constp,
            tc.tile_pool(name="drv", bufs=1) as drvp,
            tc.tile_pool(name="gmac", bufs=2) as gpool,
            tc.tile_pool(name="smac", bufs=2) as spool,
            tc.tile_pool(name="amac", bufs=2) as apool,
            tc.tile_pool(name="atr", bufs=4) as atp,
            tc.tile_pool(name="stats", bufs=2) as stp,
            tc.tile_pool(name="expt", bufs=2) as expp,
            tc.tile_pool(name="outt", bufs=2) as outp,
            tc.tile_pool(name="pstr", bufs=4, space="PSUM") as ps_t,
            tc.tile_pool(name="pso", bufs=2, space="PSUM") as ps_o,
        ):
            # ---- expand packed shard rows (66 lanes) to a 128-lane pitch
            # via an SBUF bounce, then AllGather the global tables ----
            tu = dramp.tile([ROWS, W], bf16, addr_space="Shared")
            tv = dramp.tile([ROWS, W], bf16, addr_space="Shared")
            for src, gath in ((tabu, tu), (tabv, tv)):
                pk_sb = constp.tile([P, SB * PK], bf16, tag="pk_sb")
                pk3 = pk_sb[:].rearrange("p (b w) -> p b w", w=PK)
                # packed row r = b*128 + p  ->  sbuf [p, b, :]
                src_ap = bass.AP(src.tensor, src.offset,
                                 [[PK, P], [PK * P, SB], [1, PK]])
                nc.sync.dma_start(pk3, src_ap)
                exp = dramp.tile([SHARD_ROWS, W], bf16, tag="exp")
                dst_ap = bass.AP(exp.tensor, exp[:].offset,
                                 [[W, P], [W * P, SB], [1, PK]])
                nc.sync.dma_start(dst_ap, pk3)
                nc.gpsimd.collective_compute(
                    "AllGather", mybir.AluOpType.bypass,
                    replica_groups=[list(range(NCORES))],
                    ins=[exp.opt()], outs=[gath.opt()])
            u_c0 = tu[0:CHUNK, :]
            u_c1 = tu[CHUNK:ROWS, :]
            v_c0 = tv[0:CHUNK, :]
            v_c1 = tv[CHUNK:ROWS, :]

            # ---- derive per-chunk int16 gather indices from the uint16
            # global row id, then replicate to 128 partitions ----
            idx_u0 = constp.tile([P, NMACRO * P], i16)
            idx_u1 = constp.tile([P, NMACRO * P], i16)
            idx_v0 = constp.tile([P, NMACRO * P], i16)
            idx_v1 = constp.tile([P, NMACRO * P], i16)
            NSL = 8
            SL = NMACRO * P // NSL  # 800
            for jsrc, i0_t, i1_t in ((ju, idx_u0, idx_u1),
                                     (jv, idx_v0, idx_v1)):
                for sl in range(NSL):
                    csl = slice(sl * SL, (sl + 1) * SL)
                    jst = drvp.tile([16, SL], i16, tag="jst")
                    nc.sync.dma_start(jst[:], jsrc[:, csl])
                    jf = drvp.tile([16, SL], f32, tag="jf")
                    m = drvp.tile([16, SL], f32, tag="m")
                    jm = drvp.tile([16, SL], f32, tag="jm")
                    t = drvp.tile([16, SL], f32, tag="t")
                    nc.scalar.copy(jf[:], jst[:])
                    # n = jf + 65536*(jf<0);  t = n - SPLIT
                    nc.vector.tensor_scalar(m[:], jf[:], 0.0, None, Alu.is_lt)
                    nc.vector.tensor_scalar(jm[:], jf[:], float(-SPLIT), None,
                                            Alu.add)
                    nc.vector.scalar_tensor_tensor(t[:], m[:], 65536.0, jm[:],
                                                   Alu.mult, Alu.add)
                    # chunk0: t<0 -> n, else DUMMY:  min(t,0)+DUMMY
                    nc.vector.tensor_scalar(jf[:], t[:], 0.0, float(DUMMY),
                                            Alu.min, Alu.add)
                    s0 = drvp.tile([16, SL], i16, tag="s0")
                    nc.scalar.copy(s0[:], jf[:])
                    # chunk1: t>=0 -> t, else DUMMY: (t-DUMMY)*[t>=0]+DUMMY
                    nc.vector.tensor_scalar(m[:], t[:], 0.0, None, Alu.is_ge)
                    nc.vector.scalar_tensor_tensor(jm[:], t[:], float(DUMMY),
                                                   m[:], Alu.subtract,
                                                   Alu.mult)
                    nc.vector.tensor_scalar(jm[:], jm[:], float(DUMMY), None,
                                            Alu.add)
                    s1 = drvp.tile([16, SL], i16, tag="s1")
                    nc.scalar.copy(s1[:], jm[:])
                    for j in range(8):
                        nc.sync.dma_start(i0_t[16 * j:16 * (j + 1), csl],
                                          s0[:])
                        nc.sync.dma_start(i1_t[16 * j:16 * (j + 1), csl],
                                          s1[:])

            w2a_t = constp.tile([AW, 16], f32)
            nc.sync.dma_start(w2a_t[:], w2a)
            cet_t = constp.tile([P, TYPES_PER_CORE * 64], f32)
            nc.sync.dma_start(cet_t[:], cetrow)
            b0_t = constp.tile([P, 64], f32)
            nc.sync.dma_start(b0_t[:], b0row)
            id_t = constp.tile([P, P], f32)
            make_identity(nc, id_t[:])

            def mid_bc(ap2, n):
                (ps, pc), (fs, fc) = ap2.ap
                return bass.AP(ap2.tensor, ap2.offset,
                               [[ps, pc], [0, n], [fs, fc]])

            def bc(ap2, n):
                return bass.AP(ap2.tensor, ap2.offset,
                               list(ap2.ap) + [[0, n]])

            b0_bc3 = mid_bc(b0_t[:], G)

            CH = 1024
            for m in range(NMACRO):
                tloc = m // TMACRO

                gu0 = gpool.tile([P, G * W], bf16, tag="gu0")
                gu1 = gpool.tile([P, G * W], bf16, tag="gu1")
                gv0 = gpool.tile([P, G * W], bf16, tag="gv0")
                gv1 = gpool.tile([P, G * W], bf16, tag="gv1")
                for gt, chunk_ap, idxt in ((gu0, u_c0, idx_u0),
                                           (gu1, u_c1, idx_u1),
                                           (gv0, v_c0, idx_v0),
                                           (gv1, v_c1, idx_v1)):
                    g3 = gt[:].rearrange("p (g w) -> p g w", w=W)
                    for k0 in range(0, EDGES_PER_MACRO, CH):
                        g0 = k0 // P
                        gn = CH // P
                        isl = slice(m * P + k0 // 16, m * P + (k0 + CH) // 16)
                        nc.gpsimd.dma_gather(
                            g3[:, g0:g0 + gn, :], chunk_ap, idxt[:, isl],
                            CH, CH, W, single_packet=False)

                # ---- sum the 4 gathered pieces (chunk dummies are zero) ----
                su = spool.tile([P, G * W], f32, tag="su")
                sv = spool.tile([P, G * W], f32, tag="sv", bufs=1)
                nc.vector.tensor_tensor(su[:], gu0[:], gu1[:], Alu.add)
                nc.vector.tensor_tensor(sv[:], gv0[:], gv1[:], Alu.add)
                nc.vector.tensor_tensor(su[:], su[:], sv[:], Alu.add)
                su3 = su[:].rearrange("p (g w) -> p g w", w=W)

                # ---- per-edge LayerNorm rstd from the stats lanes ----
                S1 = su3[:, :, 64:65]
                S2 = su3[:, :, 65:66]
                m_a = stp.tile([P, G], f32, tag="m_a")
                q_a = stp.tile([P, G], f32, tag="q_a")
                m_a3 = m_a[:].rearrange("p (g one) -> p g one", one=1)
                q_a3 = q_a[:].rearrange("p (g one) -> p g one", one=1)
                nc.vector.tensor_scalar(m_a3, S1, 1.0 / TOTAL_IN,
                                        3.0 / TOTAL_IN, Alu.mult, Alu.add)
                nc.vector.tensor_scalar(q_a3, S2, 1.0 / TOTAL_IN,
                                        3.0 / TOTAL_IN + EPS, Alu.mult,
                                        Alu.add)
                nc.vector.tensor_tensor(m_a[:], m_a[:], m_a[:], Alu.mult)
                nc.vector.tensor_tensor(q_a[:], q_a[:], m_a[:], Alu.subtract)
                rstd = stp.tile([P, G], f32, tag="rstd")
                nc.scalar.sqrt(rstd[:], q_a[:])
                nc.vector.reciprocal(rstd[:], rstd[:])

                # ---- a = relu(rstd * (g + cet) + b0), ones lane for bias ----
                cet_bc3 = mid_bc(cet_t[:, tloc * 64:(tloc + 1) * 64], G)
                a = apool.tile([P, G * AW], f32)
                a3 = a[:].rearrange("p (g w) -> p g w", w=AW)
                av = a3[:, :, 0:64]
                nc.vector.tensor_tensor(av, su3[:, :, 0:64], cet_bc3, Alu.add)
                nc.vector.tensor_tensor(av, av, bc(rstd[:], 64), Alu.mult)
                nc.vector.tensor_tensor(av, av, b0_bc3, Alu.add)
                nc.vector.memset(a3[:, :, 64], 1.0)
                nc.scalar.activation(av, av, Act.Relu)

                # ---- per group: PE transpose, copy, W2 matmul ----
                ops = ps_o.tile([P, G * 16], f32)
                for gi in range(G):
                    at_ps = ps_t.tile([AW, P], f32)
                    nc.tensor.transpose(at_ps[:], a3[:, gi, :], id_t[:])
                    at_sb = atp.tile([AW, P], f32)
                    nc.scalar.copy(at_sb[:], at_ps[:])
                    nc.tensor.matmul(ops[:, gi * 16:(gi + 1) * 16],
                                     lhsT=at_sb[:], rhs=w2a_t[:],
                                     start=True, stop=True)

                # ---- batched softmax tail: ship att*255 as uint8 ----
                ex = expp.tile([P, G * 16], f32)
                nc.scalar.activation(ex[:], ops[:], Act.Exp)
                ex3 = ex[:].rearrange("p (r w) -> p r w", w=4)
                sums = stp.tile([P, 4 * G], f32, tag="sums")
                nc.vector.tensor_reduce(sums[:], ex3, mybir.AxisListType.X,
                                        Alu.add)
                rec = stp.tile([P, 4 * G], f32, tag="rec")
                nc.vector.reciprocal(rec[:], sums[:])
                nc.vector.tensor_scalar_mul(rec[:], rec[:], OSCALE)
                ot = outp.tile([P, G * 16], f32)
                ot3 = ot[:].rearrange("p (r w) -> p r w", w=4)
                nc.vector.tensor_tensor(ot3, ex3, bc(rec[:], 4), Alu.mult)
                oi = outp.tile([P, G * 16], u8, tag="oi")
                nc.scalar.copy(oi[:], ot[:])
                nc.sync.dma_start(out_d[m], oi[:])

    nc.compile()
    return nc


def _get_runtime():
    if "rt" in _CACHE:
        return _CACHE["rt"]

    import jax
    import jax.numpy as jnp
    from jax.sharding import Mesh, PartitionSpec, NamedSharding
    from jax.experimental.shard_map import shard_map
    import concourse.bass2jax as b2j
    import concourse.mybir as mybir

    nc = _build_program()
    b2j.install_neuronx_cc_hook()

    partition_name = (nc.partition_id_tensor.name
                      if nc.partition_id_tensor else None)
    in_names, out_names, out_avals = [], [], []
    for alloc in nc.m.functions[0].allocations:
        if not isinstance(alloc, mybir.MemoryLocationSet):
            continue
        name = alloc.memorylocations[0].name
        if alloc.kind == "ExternalInput":
            if name != partition_name:
                in_names.append(name)
        elif alloc.kind == "ExternalOutput":
            assert alloc.tensor_shape is not None and alloc.dtype is not None
            out_names.append(name)
            out_avals.append(jax.core.ShapedArray(
                tuple(alloc.tensor_shape), mybir.dt.np(alloc.dtype)))
    n_params = len(in_names)
    n_outs = len(out_avals)
    in_names_all = in_names + out_names
    if partition_name is not None:
        in_names_all = in_names_all + [partition_name]
    donate = tuple(range(n_params, n_params + n_outs))

    def _body(*args):
        operands = list(args)
        if partition_name is not None:
            operands.append(b2j.partition_id_tensor())
        outs = b2j._bass_exec_p.bind(
            *operands, out_avals=tuple(out_avals),
            in_names=tuple(in_names_all), out_names=tuple(out_names),
            lowering_input_output_aliases=(), sim_require_finite=True,
            sim_require_nnan=True, nc=nc)
        return tuple(outs)

    devices = jax.devices()[:NCORES]
    assert len(devices) == NCORES
    mesh = Mesh(np.asarray(devices), ("core",))
    in_specs = (PartitionSpec("core"),) * (n_params + n_outs)
    out_specs = (PartitionSpec("core"),) * n_outs
    sharded = jax.jit(
        shard_map(_body, mesh=mesh, in_specs=in_specs, out_specs=out_specs,
                  check_rep=False),
        donate_argnums=donate, keep_unused=True)

    zshapes = [(NCORES * a.shape[0], *a.shape[1:]) for a in out_avals]
    zdtypes = [a.dtype for a in out_avals]
    csharding = NamedSharding(mesh, PartitionSpec("core"))

    def _zeros():
        return tuple(jnp.zeros(s, d) for s, d in zip(zshapes, zdtypes))

    zeros_fn = jax.jit(_zeros,
                       out_shardings=tuple(csharding for _ in zshapes))

    rt = {"nc": nc, "sharded": sharded, "zeros_fn": zeros_fn,
          "in_names": in_names, "out_names": out_names,
          "csharding": csharding, "jax": jax, "devices": list(devices)}
    _CACHE["rt"] = rt
    return rt


def _seq_to_pat(vals):
    """Edge-seq-ordered int values -> the dma_gather wrapped index layout.

    seq slot (m, p, g) = m*2048 + p*16 + g; gather list pos i = g*128 + p;
    idx sbuf column layout pat[p16, m*128 + s] = list[s*16 + p16].
    """
    v = vals.reshape(NMACRO, P, G).transpose(0, 2, 1).reshape(NMACRO, 2048)
    pat = v.reshape(NMACRO, P, 16).transpose(0, 2, 1)  # [NMACRO, 16, P]
    return np.ascontiguousarray(
        pat.transpose(1, 0, 2).reshape(16, NMACRO * P)).astype(np.int16)


def _shard_node_range(c):
    """Global table rows [c*6400, (c+1)*6400) -> (node_lo, node_hi)."""
    r0, r1 = c * SHARD_ROWS, (c + 1) * SHARD_ROWS
    if r1 <= CHUNK:  # chunk0 rows map to nodes [0, SPLIT) then zeros
        return min(r0, SPLIT), min(r1, SPLIT)
    n0 = SPLIT + (r0 - CHUNK)
    return min(n0, N), min(n0 + SHARD_ROWS, N)


def _prep_tables_pipelined(x, node_types, ln_w, ln_b, W1, b1, W2, b2,
                           put_shard):
    """Build the u/v node tables in per-core slabs, handing each slab to
    put_shard(name, core, slab) as soon as it exists so the h2d transfer
    overlaps the remaining host work."""
    import ml_dtypes
    bf16 = ml_dtypes.bfloat16

    x = np.asarray(x, np.float32)
    ln_w = np.asarray(ln_w, np.float32)
    ln_b = np.asarray(ln_b, np.float32)
    W1 = np.asarray(W1, np.float32)
    b1 = np.asarray(b1, np.float32)
    W2 = np.asarray(W2, np.float32)
    b2 = np.asarray(b2, np.float32)

    W1p = ln_w[:, None] * W1
    s = W1p.sum(0)
    b0 = b1 + ln_b @ W1
    AB = W1p[0:2 * C]  # [256, 64]: A on top, B below
    C1 = W1p[2 * C:2 * C + NT]
    C2 = W1p[2 * C + NT:2 * C + 2 * NT]
    Cet = W1p[2 * C + 2 * NT:]
    cet_r = (Cet - (3.0 / TOTAL_IN) * s[None, :]).astype(np.float32)

    sx = x.sum(1)
    sqx = (x * x).sum(1)
    nt = np.asarray(node_types).astype(np.int64)

    M2 = np.ascontiguousarray(np.concatenate([AB[0:C], AB[C:2 * C]], 1))
    for c in range(NCORES):
        n0, n1 = _shard_node_range(c)
        cnt = n1 - n0
        slab_u = np.zeros((SHARD_ROWS, PK), bf16)
        slab_v = np.zeros((SHARD_ROWS, PK), bf16)
        if cnt > 0:
            xs = x[n0:n1]
            uv = xs @ M2  # [cnt, 128] = [u64 | v64]
            mu = (sx[n0:n1] / TOTAL_IN)[:, None] * s[None, :]
            nts = nt[n0:n1]
            fu = np.empty((cnt, PK), np.float32)
            fu[:, 0:64] = uv[:, 0:64] + C1[nts] - mu
            fu[:, 64] = sx[n0:n1]
            fu[:, 65] = sqx[n0:n1]
            slab_u[0:cnt] = fu
            fu[:, 0:64] = uv[:, 64:128] + C2[nts] - mu
            slab_v[0:cnt] = fu
        put_shard("tabu", c, slab_u)
        put_shard("tabv", c, slab_v)

    w2a = np.concatenate([W2, b2[None, :]], 0).astype(np.float32)
    b0row = np.tile(b0[None, :].astype(np.float32), (P, 1))
    return cet_r, w2a, b0row


def _prep_indices(edge_index, edge_types, cet_r):
    row = np.asarray(edge_index[0]).astype(np.int64)
    col = np.asarray(edge_index[1]).astype(np.int64)
    et = np.asarray(edge_types).astype(np.int64)

    order = np.argsort(et, kind="stable")
    counts = np.bincount(et, minlength=ET)
    assert counts.max() <= E_TYPE_PAD, counts.max()
    starts = np.zeros(ET + 1, np.int64)
    np.cumsum(counts, out=starts[1:])

    per_core = {k: [] for k in ("ju", "jv", "cetrow")}
    unscatter = []
    for c in range(NCORES):
        seq = np.zeros(E_PAD, np.int64)
        un = []
        for k in range(TYPES_PER_CORE):
            t = c * TYPES_PER_CORE + k
            ids = order[starts[t]:starts[t + 1]]
            seq[k * E_TYPE_PAD:k * E_TYPE_PAD + len(ids)] = ids
            un.append((ids, k))
        unscatter.append(un)

        per_core["ju"].append(_seq_to_pat(row[seq]))
        per_core["jv"].append(_seq_to_pat(col[seq]))
        per_core["cetrow"].append(np.tile(
            cet_r[c * TYPES_PER_CORE:(c + 1) * TYPES_PER_CORE].reshape(
                1, TYPES_PER_CORE * 64), (P, 1)).astype(np.float32))

    concat = {k: np.concatenate(v, 0) for k, v in per_core.items()}
    return concat, unscatter


def kernel(**inputs) -> np.ndarray:
    global LAST_RESULTS
    LAST_RESULTS = None

    rt = _get_runtime()
    jax = rt["jax"]
    devices = rt["devices"]
    put = lambda a: jax.device_put(a, rt["csharding"])

    zeros = rt["zeros_fn"]()  # async, on-device

    # build tables in per-core slabs; each slab's h2d starts immediately
    # and overlaps the remaining host prep
    slab_parts = {"tabu": [None] * NCORES, "tabv": [None] * NCORES}

    def put_shard(name, c, slab):
        slab_parts[name][c] = jax.device_put(slab, devices[c])

    cet_r, w2a, b0row = _prep_tables_pipelined(
        **{k: inputs[k] for k in ("x", "node_types", "ln_w", "ln_b",
                                  "W1", "b1", "W2", "b2")},
        put_shard=put_shard)
    dev = {
        name: jax.make_array_from_single_device_arrays(
            (NCORES * SHARD_ROWS, PK), rt["csharding"], parts)
        for name, parts in slab_parts.items()
    }

    concat, unscatter = _prep_indices(inputs["edge_index"],
                                      inputs["edge_types"], cet_r)
    dev.update({k: put(v) for k, v in concat.items()})
    dev["b0row"] = put(np.tile(b0row, (NCORES, 1)))
    dev["w2a"] = put(np.tile(w2a, (NCORES, 1)))

    args = [dev[name] for name in rt["in_names"]] + list(zeros)
    out_arrs = rt["sharded"](*args)

    out = out_arrs[0]  # [NCORES*NMACRO, P, G*16] uint8, sharded
    shards = sorted(out.addressable_shards, key=lambda s: s.index[0].start)
    datas = [s.data for s in shards]
    for dd in datas:
        dd.copy_to_host_async()

    # shipped q = round(255*att); output = I - q/255
    full = np.empty((E, 16), np.float32)
    minv = np.float32(-1.0 / OSCALE)
    for c in range(NCORES):
        rows = np.asarray(datas[c]).reshape(E_PAD, 16)
        for ids, k in unscatter[c]:
            full[ids] = rows[k * E_TYPE_PAD:k * E_TYPE_PAD + len(ids)]
    full *= minv
    full[:, 0] += 1.0
    full[:, 5] += 1.0
    full[:, 10] += 1.0
    full[:, 15] += 1.0
    return full.reshape(E, D, D)


# revision 14
# speedup vs baseline: 27.5959x; 1.2364x over previous
"""v6: wire-optimized edge-parallel sheaf learner.

End-to-end wall time is dominated by host<->device transfer over the
axon tunnel (~50 MB/s), not on-device compute. v3 shipped ~302 MB per
call; v6 ships ~18 MB in + 13 MB out:

- Per-node tables (u64|sx|sqx bf16) ship PACKED [rows, 66] and SHARDED
  1/8 per core; each core expands its shard to 128-wide rows (the
  256B-aligned row pitch dma_gather requires) via an SBUF bounce, then
  an on-device AllGather over NeuronLink assembles the global tables.
- dma_gather indices are int16-limited (<32768), so the 51200-row table
  is split in two 25600-row chunks with zero-filled dummy rows; each
  edge gathers from both chunks (out-of-chunk index -> zero row) and
  the two results are summed. No masks needed.
- Only ONE uint16 global row-id per edge side ships (as [16, X], the
  native wrapped layout); both per-chunk int16 index tiles are derived
  on-device in f32 (exact for ints < 2^24) and replicated to 128
  partitions with SBUF DMAs.
- The softmax ships as uint8 att*255 (f32->int cast is round-to-nearest
  -even, verified on HW); the host reconstructs I - att. Quantization
  error ~0.2% of output scale vs the 2e-2 gate.
- The jitted shard_map executable is cached across calls; donated
  output buffers are zero-filled ON DEVICE; big inputs are device_put
  asynchronously as soon as host prep produces them; d2h is fetched
  per-shard and overlapped with the host-side unscatter.
"""

import numpy as np

N, E = 50000, 800000
C, NT, ET, H, D = 128, 8, 16, 64, 4
TOTAL_IN = 2 * C + 2 * NT + ET  # 288
EPS = 1e-5

P = 128
G = 16
EDGES_PER_MACRO = P * G     # 2048
NCORES = 8
TYPES_PER_CORE = ET // NCORES   # 2
TMACRO = 25
NMACRO = TYPES_PER_CORE * TMACRO  # 50
E_TYPE_PAD = TMACRO * EDGES_PER_MACRO   # 51200
E_PAD = NMACRO * EDGES_PER_MACRO        # 102400
AW = 65                     # a | ones

W = 128                     # table row width in HBM (bf16) = 256B
PK = 66                     # packed row width on the wire: u64|sx|sqx
SPLIT = 25000               # nodes per chunk
CHUNK = 25600               # rows per chunk (600 zero rows at the end)
DUMMY = 25000               # local zero-row index in each chunk
ROWS = 2 * CHUNK            # 51200 = 8 * 6400
SHARD_ROWS = ROWS // NCORES  # 6400
SB = SHARD_ROWS // P        # 50 row-blocks per shard

OSCALE = 255.0

_CACHE = {}
LAST_RESULTS = None


def _build_program():
    import concourse.bacc as bacc
    import concourse.bass as bass
    import concourse.tile as tile
    import concourse.mybir as mybir
    from concourse.masks import make_identity

    f32 = mybir.dt.float32
    bf16 = mybir.dt.bfloat16
    i16 = mybir.dt.int16
    u8 = mybir.dt.uint8
    Alu = mybir.AluOpType
    Act = mybir.ActivationFunctionType

    nc = bacc.Bacc("TRN2", target_bir_lowering=False, debug=False,
                   num_devices=NCORES, dynamic_dma_scratch_size=65536)

    tabu = nc.dram_tensor("tabu", [SHARD_ROWS, PK], bf16,
                          kind="ExternalInput").ap()
    tabv = nc.dram_tensor("tabv", [SHARD_ROWS, PK], bf16,
                          kind="ExternalInput").ap()
    ju = nc.dram_tensor("ju", [16, NMACRO * P], i16,
                        kind="ExternalInput").ap()
    jv = nc.dram_tensor("jv", [16, NMACRO * P], i16,
                        kind="ExternalInput").ap()
    cetrow = nc.dram_tensor("cetrow", [P, TYPES_PER_CORE * 64], f32,
                            kind="ExternalInput").ap()
    b0row = nc.dram_tensor("b0row", [P, 64], f32, kind="ExternalInput").ap()
    w2a = nc.dram_tensor("w2a", [AW, 16], f32, kind="ExternalInput").ap()
    out_d = nc.dram_tensor("out", [NMACRO, P, G * 16], u8,
                           kind="ExternalOutput").ap()

    with tile.TileContext(nc) as tc:
        with (
            tc.tile_pool(name="dram", bufs=1, space="DRAM") as dramp,
            tc.tile_pool(name="const", bufs=1) as constp,
            tc.tile_pool(name="drv", bufs=1) as drvp,
            tc.tile_pool(name="gmac", bufs=2) as gpool,
            tc.tile_pool(name="smac", bufs=2) as spool,
            tc.tile_pool(name="amac", bufs=2) as apool,
            tc.tile_pool(name="atr", bufs=4) as atp,
            tc.tile_pool(name="stats", bufs=2) as stp,
            tc.tile_pool(name="expt", bufs=2) as expp,
            tc.tile_pool(name="outt", bufs=2) as outp,
            tc.tile_pool(name="pstr", bufs=4, space="PSUM") as ps_t,
            tc.tile_pool(name="pso", bufs=2, space="PSUM") as ps_o,
        ):
            # ---- expand packed shard rows (66 lanes) to a 128-lane pitch
            # via an SBUF bounce, then AllGather the global tables ----
            tu = dramp.tile([ROWS, W], bf16, addr_space="Shared")
            tv = dramp.tile([ROWS, W], bf16, addr_space="Shared")
            for src, gath in ((tabu, tu), (tabv, tv)):
                pk_sb = constp.tile([P, SB * PK], bf16, tag="pk_sb")
                pk3 = pk_sb[:].rearrange("p (b w) -> p b w", w=PK)
                # packed row r = b*128 + p  ->  sbuf [p, b, :]
                src_ap = bass.AP(src.tensor, src.offset,
                                 [[PK, P], [PK * P, SB], [1, PK]])
                nc.sync.dma_start(pk3, src_ap)
                exp = dramp.tile([SHARD_ROWS, W], bf16, tag="exp")
                dst_ap = bass.AP(exp.tensor, exp[:].offset,
                                 [[W, P], [W * P, SB], [1, PK]])
                nc.sync.dma_start(dst_ap, pk3)
                nc.gpsimd.collective_compute(
                    "AllGather", mybir.AluOpType.bypass,
                    replica_groups=[list(range(NCORES))],
                    ins=[exp.opt()], outs=[gath.opt()])
            u_c0 = tu[0:CHUNK, :]
            u_c1 = tu[CHUNK:ROWS, :]
            v_c0 = tv[0:CHUNK, :]
            v_c1 = tv[CHUNK:ROWS, :]

            # ---- derive per-chunk int16 gather indices from the uint16
            # global row id, then replicate to 128 partitions ----
            idx_u0 = constp.tile([P, NMACRO * P], i16)
            idx_u1 = constp.tile([P, NMACRO * P], i16)
            idx_v0 = constp.tile([P, NMACRO * P], i16)
            idx_v1 = constp.tile([P, NMACRO * P], i16)
            NSL = 8
            SL = NMACRO * P // NSL  # 800
            for jsrc, i0_t, i1_t in ((ju, idx_u0, idx_u1),
                                     (jv, idx_v0, idx_v1)):
                for sl in range(NSL):
                    csl = slice(sl * SL, (sl + 1) * SL)
                    jst = drvp.tile([16, SL], i16, tag="jst")
                    nc.sync.dma_start(jst[:], jsrc[:, csl])
                    jf = drvp.tile([16, SL], f32, tag="jf")
                    m = drvp.tile([16, SL], f32, tag="m")
                    jm = drvp.tile([16, SL], f32, tag="jm")
                    t = drvp.tile([16, SL], f32, tag="t")
                    nc.scalar.copy(jf[:], jst[:])
                    # n = jf + 65536*(jf<0);  t = n - SPLIT
                    nc.vector.tensor_scalar(m[:], jf[:], 0.0, None, Alu.is_lt)
                    nc.vector.tensor_scalar(jm[:], jf[:], float(-SPLIT), None,
                                            Alu.add)
                    nc.vector.scalar_tensor_tensor(t[:], m[:], 65536.0, jm[:],
                                                   Alu.mult, Alu.add)
                    # chunk0: t<0 -> n, else DUMMY:  min(t,0)+DUMMY
                    nc.vector.tensor_scalar(jf[:], t[:], 0.0, float(DUMMY),
                                            Alu.min, Alu.add)
                    s0 = drvp.tile([16, SL], i16, tag="s0")
                    nc.scalar.copy(s0[:], jf[:])
                    # chunk1: t>=0 -> t, else DUMMY: (t-DUMMY)*[t>=0]+DUMMY
                    nc.vector.tensor_scalar(m[:], t[:], 0.0, None, Alu.is_ge)
                    nc.vector.scalar_tensor_tensor(jm[:], t[:], float(DUMMY),
                                                   m[:], Alu.subtract,
                                                   Alu.mult)
                    nc.vector.tensor_scalar(jm[:], jm[:], float(DUMMY), None,
                                            Alu.add)
                    s1 = drvp.tile([16, SL], i16, tag="s1")
                    nc.scalar.copy(s1[:], jm[:])
                    for j in range(8):
                        nc.sync.dma_start(i0_t[16 * j:16 * (j + 1), csl],
                                          s0[:])
                        nc.sync.dma_start(i1_t[16 * j:16 * (j + 1), csl],
                                          s1[:])

            w2a_t = constp.tile([AW, 16], f32)
            nc.sync.dma_start(w2a_t[:], w2a)
            cet_t = constp.tile([P, TYPES_PER_CORE * 64], f32)
            nc.sync.dma_start(cet_t[:], cetrow)
            b0_t = constp.tile([P, 64], f32)
            nc.sync.dma_start(b0_t[:], b0row)
            id_t = constp.tile([P, P], f32)
            make_identity(nc, id_t[:])

            def mid_bc(ap2, n):
                (ps, pc), (fs, fc) = ap2.ap
                return bass.AP(ap2.tensor, ap2.offset,
                               [[ps, pc], [0, n], [fs, fc]])

            def bc(ap2, n):
                return bass.AP(ap2.tensor, ap2.offset,
                               list(ap2.ap) + [[0, n]])

            b0_bc3 = mid_bc(b0_t[:], G)

            CH = 1024
            for m in range(NMACRO):
                tloc = m // TMACRO

                gu0 = gpool.tile([P, G * W], bf16, tag="gu0")
                gu1 = gpool.tile([P, G * W], bf16, tag="gu1")
                gv0 = gpool.tile([P, G * W], bf16, tag="gv0")
                gv1 = gpool.tile([P, G * W], bf16, tag="gv1")
                for gt, chunk_ap, idxt in ((gu0, u_c0, idx_u0),
                                           (gu1, u_c1, idx_u1),
                                           (gv0, v_c0, idx_v0),
                                           (gv1, v_c1, idx_v1)):
                    g3 = gt[:].rearrange("p (g w) -> p g w", w=W)
                    for k0 in range(0, EDGES_PER_MACRO, CH):
                        g0 = k0 // P
                        gn = CH // P
                        isl = slice(m * P + k0 // 16, m * P + (k0 + CH) // 16)
                        nc.gpsimd.dma_gather(
                            g3[:, g0:g0 + gn, :], chunk_ap, idxt[:, isl],
                            CH, CH, W, single_packet=False)

                # ---- sum the 4 gathered pieces (chunk dummies are zero) ----
                su = spool.tile([P, G * W], f32, tag="su")
                sv = spool.tile([P, G * W], f32, tag="sv", bufs=1)
                nc.vector.tensor_tensor(su[:], gu0[:], gu1[:], Alu.add)
                nc.vector.tensor_tensor(sv[:], gv0[:], gv1[:], Alu.add)
                nc.vector.tensor_tensor(su[:], su[:], sv[:], Alu.add)
                su3 = su[:].rearrange("p (g w) -> p g w", w=W)

                # ---- per-edge LayerNorm rstd from the stats lanes ----
                S1 = su3[:, :, 64:65]
                S2 = su3[:, :, 65:66]
                m_a = stp.tile([P, G], f32, tag="m_a")
                q_a = stp.tile([P, G], f32, tag="q_a")
                m_a3 = m_a[:].rearrange("p (g one) -> p g one", one=1)
                q_a3 = q_a[:].rearrange("p (g one) -> p g one", one=1)
                nc.vector.tensor_scalar(m_a3, S1, 1.0 / TOTAL_IN,
                                        3.0 / TOTAL_IN, Alu.mult, Alu.add)
                nc.vector.tensor_scalar(q_a3, S2, 1.0 / TOTAL_IN,
                                        3.0 / TOTAL_IN + EPS, Alu.mult,
                                        Alu.add)
                nc.vector.tensor_tensor(m_a[:], m_a[:], m_a[:], Alu.mult)
                nc.vector.tensor_tensor(q_a[:], q_a[:], m_a[:], Alu.subtract)
                rstd = stp.tile([P, G], f32, tag="rstd")
                nc.scalar.sqrt(rstd[:], q_a[:])
                nc.vector.reciprocal(rstd[:], rstd[:])

                # ---- a = relu(rstd * (g + cet) + b0), ones lane for bias ----
                cet_bc3 = mid_bc(cet_t[:, tloc * 64:(tloc + 1) * 64], G)
                a = apool.tile([P, G * AW], f32)
                a3 = a[:].rearrange("p (g w) -> p g w", w=AW)
                av = a3[:, :, 0:64]
                nc.vector.tensor_tensor(av, su3[:, :, 0:64], cet_bc3, Alu.add)
                nc.vector.tensor_tensor(av, av, bc(rstd[:], 64), Alu.mult)
                nc.vector.tensor_tensor(av, av, b0_bc3, Alu.add)
                nc.vector.memset(a3[:, :, 64], 1.0)
                nc.scalar.activation(av, av, Act.Relu)

                # ---- per group: PE transpose, copy, W2 matmul ----
                ops = ps_o.tile([P, G * 16], f32)
                for gi in range(G):
                    at_ps = ps_t.tile([AW, P], f32)
                    nc.tensor.transpose(at_ps[:], a3[:, gi, :], id_t[:])
                    at_sb = atp.tile([AW, P], f32)
                    nc.scalar.copy(at_sb[:], at_ps[:])
                    nc.tensor.matmul(ops[:, gi * 16:(gi + 1) * 16],
                                     lhsT=at_sb[:], rhs=w2a_t[:],
                                     start=True, stop=True)

                # ---- batched softmax tail: ship att*255 as uint8 ----
                ex = expp.tile([P, G * 16], f32)
                nc.scalar.activation(ex[:], ops[:], Act.Exp)
                ex3 = ex[:].rearrange("p (r w) -> p r w", w=4)
                sums = stp.tile([P, 4 * G], f32, tag="sums")
                nc.vector.tensor_reduce(sums[:], ex3, mybir.AxisListType.X,
                                        Alu.add)
                rec = stp.tile([P, 4 * G], f32, tag="rec")
                nc.vector.reciprocal(rec[:], sums[:])
                nc.vector.tensor_scalar_mul(rec[:], rec[:], OSCALE)
                ot = outp.tile([P, G * 16], f32)
                ot3 = ot[:].rearrange("p (r w) -> p r w", w=4)
                nc.vector.tensor_tensor(ot3, ex3, bc(rec[:], 4), Alu.mult)
                oi = outp.tile([P, G * 16], u8, tag="oi")
                nc.scalar.copy(oi[:], ot[:])
                nc.sync.dma_start(out_d[m], oi[:])

    nc.compile()
    return nc


def _get_runtime():
    if "rt" in _CACHE:
        return _CACHE["rt"]

    import jax
    import jax.numpy as jnp
    from jax.sharding import Mesh, PartitionSpec, NamedSharding
    from jax.experimental.shard_map import shard_map
    import concourse.bass2jax as b2j
    import concourse.mybir as mybir

    nc = _build_program()
    b2j.install_neuronx_cc_hook()

    partition_name = (nc.partition_id_tensor.name
                      if nc.partition_id_tensor else None)
    in_names, out_names, out_avals = [], [], []
    for alloc in nc.m.functions[0].allocations:
        if not isinstance(alloc, mybir.MemoryLocationSet):
            continue
        name = alloc.memorylocations[0].name
        if alloc.kind == "ExternalInput":
            if name != partition_name:
                in_names.append(name)
        elif alloc.kind == "ExternalOutput":
            assert alloc.tensor_shape is not None and alloc.dtype is not None
            out_names.append(name)
            out_avals.append(jax.core.ShapedArray(
                tuple(alloc.tensor_shape), mybir.dt.np(alloc.dtype)))
    n_params = len(in_names)
    n_outs = len(out_avals)
    in_names_all = in_names + out_names
    if partition_name is not None:
        in_names_all = in_names_all + [partition_name]
    donate = tuple(range(n_params, n_params + n_outs))

    def _body(*args):
        operands = list(args)
        if partition_name is not None:
            operands.append(b2j.partition_id_tensor())
        outs = b2j._bass_exec_p.bind(
            *operands, out_avals=tuple(out_avals),
            in_names=tuple(in_names_all), out_names=tuple(out_names),
            lowering_input_output_aliases=(), sim_require_finite=True,
            sim_require_nnan=True, nc=nc)
        return tuple(outs)

    devices = jax.devices()[:NCORES]
    assert len(devices) == NCORES
    mesh = Mesh(np.asarray(devices), ("core",))
    in_specs = (PartitionSpec("core"),) * (n_params + n_outs)
    out_specs = (PartitionSpec("core"),) * n_outs
    sharded = jax.jit(
        shard_map(_body, mesh=mesh, in_specs=in_specs, out_specs=out_specs,
                  check_rep=False),
        donate_argnums=donate, keep_unused=True)

    zshapes = [(NCORES * a.shape[0], *a.shape[1:]) for a in out_avals]
    zdtypes = [a.dtype for a in out_avals]
    csharding = NamedSharding(mesh, PartitionSpec("core"))

    def _zeros():
        return tuple(jnp.zeros(s, d) for s, d in zip(zshapes, zdtypes))

    zeros_fn = jax.jit(_zeros,
                       out_shardings=tuple(csharding for _ in zshapes))

    rt = {"nc": nc, "sharded": sharded, "zeros_fn": zeros_fn,
          "in_names": in_names, "out_names": out_names,
          "csharding": csharding, "jax": jax, "devices": list(devices)}
    _CACHE["rt"] = rt
    return rt


def _seq_to_pat(vals):
    """Edge-seq-ordered int values -> the dma_gather wrapped index layout.

    seq slot (m, p, g) = m*2048 + p*16 + g; gather list pos i = g*128 + p;
    idx sbuf column layout pat[p16, m*128 + s] = list[s*16 + p16].
    """
    v = vals.reshape(NMACRO, P, G).transpose(0, 2, 1).reshape(NMACRO, 2048)
    pat = v.reshape(NMACRO, P, 16).transpose(0, 2, 1)  # [NMACRO, 16, P]
    return np.ascontiguousarray(
        pat.transpose(1, 0, 2).reshape(16, NMACRO * P)).astype(np.int16)


def _shard_node_range(c):
    """Global table rows [c*6400, (c+1)*6400) -> (node_lo, node_hi)."""
    r0, r1 = c * SHARD_ROWS, (c + 1) * SHARD_ROWS
    if r1 <= CHUNK:  # chunk0 rows map to nodes [0, SPLIT) then zeros
        return min(r0, SPLIT), min(r1, SPLIT)
    n0 = SPLIT + (r0 - CHUNK)
    return min(n0, N), min(n0 + SHARD_ROWS, N)


def _prep_tables_pipelined(x, node_types, ln_w, ln_b, W1, b1, W2, b2,
                           put_shard):
    """Build the u/v node tables in per-core slabs, handing each slab to
    put_shard(name, core, slab) as soon as it exists so the h2d transfer
    overlaps the remaining host work."""
    import ml_dtypes
    bf16 = ml_dtypes.bfloat16

    x = np.asarray(x, np.float32)
    ln_w = np.asarray(ln_w, np.float32)
    ln_b = np.asarray(ln_b, np.float32)
    W1 = np.asarray(W1, np.float32)
    b1 = np.asarray(b1, np.float32)
    W2 = np.asarray(W2, np.float32)
    b2 = np.asarray(b2, np.float32)

    W1p = ln_w[:, None] * W1
    s = W1p.sum(0)
    b0 = b1 + ln_b @ W1
    AB = W1p[0:2 * C]  # [256, 64]: A on top, B below
    C1 = W1p[2 * C:2 * C + NT]
    C2 = W1p[2 * C + NT:2 * C + 2 * NT]
    Cet = W1p[2 * C + 2 * NT:]
    cet_r = (Cet - (3.0 / TOTAL_IN) * s[None, :]).astype(np.float32)

    nt = np.asarray(node_types).astype(np.int64)

    M2 = np.ascontiguousarray(np.concatenate([AB[0:C], AB[C:2 * C]], 1))
    for c in range(NCORES):
        n0, n1 = _shard_node_range(c)
        cnt = n1 - n0
        slab_u = np.zeros((SHARD_ROWS, PK), bf16)
        slab_v = np.zeros((SHARD_ROWS, PK), bf16)
        if cnt > 0:
            xs = x[n0:n1]
            sxs = xs.sum(1)
            sqxs = (xs * xs).sum(1)
            uv = xs @ M2  # [cnt, 128] = [u64 | v64]
            mu = (sxs / TOTAL_IN)[:, None] * s[None, :]
            nts = nt[n0:n1]
            fu = np.empty((cnt, PK), np.float32)
            fu[:, 0:64] = uv[:, 0:64] + C1[nts] - mu
            fu[:, 64] = sxs
            fu[:, 65] = sqxs
            slab_u[0:cnt] = fu
            fu[:, 0:64] = uv[:, 64:128] + C2[nts] - mu
            slab_v[0:cnt] = fu
        put_shard("tabu", c, slab_u)
        put_shard("tabv", c, slab_v)

    w2a = np.concatenate([W2, b2[None, :]], 0).astype(np.float32)
    b0row = np.tile(b0[None, :].astype(np.float32), (P, 1))
    return cet_r, w2a, b0row


def _prep_indices(edge_index, edge_types, cet_r):
    row = np.asarray(edge_index[0]).astype(np.int64)
    col = np.asarray(edge_index[1]).astype(np.int64)
    et = np.asarray(edge_types).astype(np.int64)

    order = np.argsort(et.astype(np.uint8), kind="stable")
    counts = np.bincount(et, minlength=ET)
    assert counts.max() <= E_TYPE_PAD, counts.max()
    starts = np.zeros(ET + 1, np.int64)
    np.cumsum(counts, out=starts[1:])

    per_core = {k: [] for k in ("ju", "jv", "cetrow")}
    unscatter = []
    for c in range(NCORES):
        seq = np.zeros(E_PAD, np.int64)
        un = []
        for k in range(TYPES_PER_CORE):
            t = c * TYPES_PER_CORE + k
            ids = order[starts[t]:starts[t + 1]]
            seq[k * E_TYPE_PAD:k * E_TYPE_PAD + len(ids)] = ids
            un.append((ids, k))
        unscatter.append(un)

        per_core["ju"].append(_seq_to_pat(row[seq]))
        per_core["jv"].append(_seq_to_pat(col[seq]))
        per_core["cetrow"].append(np.tile(
            cet_r[c * TYPES_PER_CORE:(c + 1) * TYPES_PER_CORE].reshape(
                1, TYPES_PER_CORE * 64), (P, 1)).astype(np.float32))

    concat = {k: np.concatenate(v, 0) for k, v in per_core.items()}
    return concat, unscatter


def kernel(**inputs) -> np.ndarray:
    global LAST_RESULTS
    LAST_RESULTS = None

    rt = _get_runtime()
    jax = rt["jax"]
    devices = rt["devices"]
    put = lambda a: jax.device_put(a, rt["csharding"])

    zeros = rt["zeros_fn"]()  # async, on-device

    # build tables in per-core slabs; each slab's h2d starts immediately
    # and overlaps the remaining host prep
    slab_parts = {"tabu": [None] * NCORES, "tabv": [None] * NCORES}

    def put_shard(name, c, slab):
        slab_parts[name][c] = jax.device_put(slab, devices[c])

    cet_r, w2a, b0row = _prep_tables_pipelined(
        **{k: inputs[k] for k in ("x", "node_types", "ln_w", "ln_b",
                                  "W1", "b1", "W2", "b2")},
        put_shard=put_shard)
    dev = {
        name: jax.make_array_from_single_device_arrays(
            (NCORES * SHARD_ROWS, PK), rt["csharding"], parts)
        for name, parts in slab_parts.items()
    }

    concat, unscatter = _prep_indices(inputs["edge_index"],
                                      inputs["edge_types"], cet_r)
    dev.update({k: put(v) for k, v in concat.items()})
    dev["b0row"] = put(np.tile(b0row, (NCORES, 1)))
    dev["w2a"] = put(np.tile(w2a, (NCORES, 1)))

    args = [dev[name] for name in rt["in_names"]] + list(zeros)
    out_arrs = rt["sharded"](*args)

    out = out_arrs[0]  # [NCORES*NMACRO, P, G*16] uint8, sharded
    shards = sorted(out.addressable_shards, key=lambda s: s.index[0].start)
    datas = [s.data for s in shards]
    for dd in datas:
        dd.copy_to_host_async()

    # shipped q = round(255*att); output = I - q/255
    full = np.empty((E, 16), np.float32)
    minv = np.float32(-1.0 / OSCALE)
    for c in range(NCORES):
        rows = np.asarray(datas[c]).reshape(E_PAD, 16)
        for ids, k in unscatter[c]:
            full[ids] = rows[k * E_TYPE_PAD:k * E_TYPE_PAD + len(ids)]
    full *= minv
    full[:, 0] += 1.0
    full[:, 5] += 1.0
    full[:, 10] += 1.0
    full[:, 15] += 1.0
    return full.reshape(E, D, D)
